# revision 20
# baseline (speedup 1.0000x reference)
"""Color-loss kernel for Trainium2 (8 NeuronCores, data-parallel over batch).

Computes, for real/fake [32, 3, 512, 512] fp32 RGB images:
    y = mean(|Y(real) - Y(fake)|)            (L1 on Y)
    u = mean(smooth_l1(U(real) - U(fake)))   (SmoothL1, beta=1)
    v = mean(smooth_l1(V(real) - V(fake)))
    loss = y + u + v
where (Y,U,V) = RGB2YUV @ rgb per pixel (skimage matrix).

Math used on-device (d := real - fake per channel; the transform is linear):
    tY2 = (dR*(RY/GY) + dG)*(GY/BY) + dB ;  dY = BY*tY2   (2 fused DVE ops)
    dU = -KU*(BY*tY2 - dB), KU = BU/(1-BY)   (row residual ~3.5e-10)
    dV = -KV*(BY*tY2 - dR), KV = RV/(1-RY)   (row residual ~1e-6; loss
        impact measured 3e-7 relative)
    smooth_l1(d) = 0.5 d^2 - 0.5 relu(|d|-1)^2
    relu(|d|-1)^2 = (ep + em - 2)^2 where ep = max(KV*t, 1), em = max(-KV*t, 1)
        (one of ep/em is always 1; s = ep+em on DVE, one ACT Square pass)
|dU| <= 0.872 < 1 always for inputs in [0,1), so U needs no relu correction;
|dV| can reach 1.23, so V keeps the correction term.

Design (measured on the 8-core axon setup; per-iteration slope timing):
  - chunk=1: one 3MB DMA per image per tensor (8 DMAs/iter). More, smaller
    pieces ("ramp"/chunk=2) cost ~1.5us per extra DMA on the steady-state
    slope metric (85us -> 67us moving ramp->chunk=1).
  - dma_split="cast": SWDGE fp32->bf16 cast during the DMA. HBM-side reads
    are unchanged (fp32), SBUF writes halve, and the DVE subtract runs in
    2x bf16 mode (3.2us vs 6.4us per image). Loss error from bf16 inputs is
    ~5e-6 relative (verified vs fp32 reference; threshold 2e-2).
  - corrfuse: the two V-correction ACT passes fold into s = ep+em (one 2x
    DVE op) + one ACT pass Square(-s/2+1) (host multiplies that accumulator
    by 4): 67us -> ~61us combined with cast.
  - abs_max (e = max(|dV|,1) in one op) has no DVE ISA encoding on TRN2;
    DMA accum_op=subtract (fold the subtract into the fake-load) is
    rejected by the BIR verifier. Deeper tile pools (io/mid bufs=3) and
    2-images-per-DMA ("pair") measured neutral-to-worse.

Engine budget per core per iteration (4 images, 24 MB HBM reads):
    DMA  ~57-60us (8x 3MB loads; ~420 GB/s/core effective = the fabric/HBM
                   roofline; this binds)
    DVE  ~44us    (bf16 subtract + 4 stt + 2 max + 1 add per image)
    ACT  ~32us    (4 accumulating activation passes per image)
ScalarE accumulates per-partition partial sums (|dY|, dU^2, dV^2, fused V
correction) into a [128, 4*G] stats tile; host sums and combines.
Measured ~60.3-60.9us/iter vs the 85.2us baseline (-29%).
"""

import os

import numpy as np

import concourse.bacc as bacc
import concourse.tile as tile
from concourse import mybir
from concourse import bass_utils

N_CORES = 8
B_FULL = 32
B_CORE = B_FULL // N_CORES  # 4 images per core
H = W = 512
PIX = H * W  # 262144 pixels per channel plane
P = 128  # SBUF partitions
FD = PIX // P  # 2048 free-dim elems per channel per image
N_PIXELS = B_FULL * PIX  # denominator of each mean

# skimage rgb2yuv matrix rows
RY, GY, BY = 0.299, 0.587, 0.114
RU, GU, BU = -0.14714119, -0.28886916, 0.43601035
RV, GV, BV = 0.61497657, -0.51496512, -0.10001026

S1Y = RY / GY  # dY chain:  tY1 = dR*S1Y + dG ; tY2 = tY1*S2Y + dB ; dY = BY*tY2
S2Y = GY / BY
KU = BU / (1.0 - BY)  # dU = -KU*(BY*tY2 - dB)   (row residual ~3.5e-10)
KV = RV / (1.0 - RY)  # dV = -KV*(BY*tY2 - dR)  (row residual ~1e-6 rel)

_CACHE = {}

# The measured-champion configuration (see module docstring). Env overrides
# exist only for local A/B experiments; unset env gives exactly this config.
DEFAULT_CHUNK = os.environ.get("KNL_CHUNK", "1")  # one 3MB DMA per image/tensor
DEFAULT_SPLIT = os.environ.get("KNL_SPLIT", "cast")  # SWDGE fp32->bf16 loads
IO_BUFS = int(os.environ.get("KNL_IOBUFS", "2"))
T_BUFS = int(os.environ.get("KNL_TBUFS", "2"))
# corrfuse: s = ep + em on DVE, one ACT pass 4*Square(-s/2+1) replaces the
# two correction passes ((ep-1)^2+(em-1)^2 == (ep+em-2)^2 since min(ep,em)=1)
CORRFUSE = os.environ.get("KNL_CORRFUSE", "1") == "1"

# accumulated quantities: |dY|, dU^2, dV^2, then either (ep-1)^2+(em-1)^2
# split over two columns, or the single fused correction column
NQ = 4 if CORRFUSE else 5


def groups_for(chunk):
    """Processing pieces as (image, j_start, j_len) over the [P, FD] plane view."""
    if chunk == "fl":
        gs = []
        for b in range(B_CORE):
            if b in (0, B_CORE - 1):
                gs += [(b, 0, FD // 2), (b, FD // 2, FD // 2)]
            else:
                gs.append((b, 0, FD))
        return gs
    if chunk == "ramp":
        q, hf = FD // 4, FD // 2
        first = [(0, 0, q), (0, q, q), (0, hf, hf)]
        last = [(B_CORE - 1, 0, hf), (B_CORE - 1, hf, q), (B_CORE - 1, hf + q, q)]
        mid = [(b, 0, FD) for b in range(1, B_CORE - 1)]
        return first + mid + last
    n = int(chunk)
    cf = FD // n
    return [(b, h * cf, cf) for b in range(B_CORE) for h in range(n)]


def _build(reps=1, mode="full", dma_split=None, chunk=None):
    """Build + compile the per-core Bass program (same SPMD program on all cores).

    reps > 1 repeats the whole computation (identical results; used by test.py
    to measure per-iteration HW time by scaling).
    mode: "full" | "dma" (loads only) | "compute" (load once, compute per rep)
    — diagnostic variants for locating the bottleneck.
    dma_split: "img" (one 3MB HWDGE DMA per image/tensor) | "cast" (SWDGE
    fp32->bf16 cast during DMA; halves SBUF write bytes and makes the DVE
    subtract run in 2x bf16 mode) | "dual" (the two loads on both HWDGE rings)
    | "plane" (one fully contiguous 1MB DMA per image/channel/tensor).
    chunk: pieces per image (1, 2, ...), "ramp", or "fl".
    """
    if dma_split is None:
        dma_split = DEFAULT_SPLIT
    if chunk is None:
        chunk = DEFAULT_CHUNK
    nc = bacc.Bacc("TRN2", target_bir_lowering=False, debug=False,
                   num_devices=N_CORES)
    f32 = mybir.dt.float32
    bf16 = mybir.dt.bfloat16
    A = mybir.AluOpType
    F = mybir.ActivationFunctionType

    groups = groups_for(chunk)  # (image, j_start, j_len) per processed piece
    G = len(groups)  # stat column groups

    real = nc.dram_tensor("real", [B_CORE, 3, H, W], f32, kind="ExternalInput").ap()
    fake = nc.dram_tensor("fake", [B_CORE, 3, H, W], f32, kind="ExternalInput").ap()
    out = nc.dram_tensor("stats", [P, NQ * G], f32, kind="ExternalOutput").ap()

    # [b, c, h, w] -> [b, p, c, j]: pixel (h, w) -> partition h//4, col (h%4)*512+w
    rview = real.rearrange("b c (p h2) w -> b p c (h2 w)", h2=4)
    fview = fake.rearrange("b c (p h2) w -> b p c (h2 w)", h2=4)
    # per-plane views [b, c, p, j] (each [p, j] slice is one contiguous 1MB range)
    rplane = real.rearrange("b c (p h2) w -> b c p (h2 w)", h2=4)
    fplane = fake.rearrange("b c (p h2) w -> b c p (h2 w)", h2=4)
    # image-pair views [bp, p, bi, c, j] for 2-images-per-DMA loading
    rpair = real.rearrange("(bp bi) c (p h2) w -> bp p bi c (h2 w)", bi=2, h2=4)
    fpair = fake.rearrange("(bp bi) c (p h2) w -> bp p bi c (h2 w)", bi=2, h2=4)

    with tile.TileContext(nc) as tc:
        with (
            tc.tile_pool(name="io", bufs=IO_BUFS) as io_pool,
            tc.tile_pool(name="dif", bufs=2) as d_pool,
            tc.tile_pool(name="mid", bufs=T_BUFS) as t_pool,
            tc.tile_pool(name="scr", bufs=2) as scr_pool,
            tc.tile_pool(name="acc", bufs=1) as s_pool,
        ):
            stats = s_pool.tile([P, NQ * G], f32)

            def load(b, j0, CF):
                dt_io = bf16 if dma_split == "cast" else f32
                rt = io_pool.tile([P, 3 * CF], dt_io, tag="rt")
                ft = io_pool.tile([P, 3 * CF], dt_io, tag="ft")
                js = slice(j0, j0 + CF)
                if dma_split == "cast":
                    nc.gpsimd.dma_start(
                        out=rt[:].rearrange("p (c j) -> p c j", c=3),
                        in_=rview[b][:, :, js],
                    )
                    nc.gpsimd.dma_start(
                        out=ft[:].rearrange("p (c j) -> p c j", c=3),
                        in_=fview[b][:, :, js],
                    )
                elif dma_split in ("img", "dual"):
                    eng_ft = nc.scalar if dma_split == "dual" else nc.sync
                    nc.sync.dma_start(
                        out=rt[:].rearrange("p (c j) -> p c j", c=3),
                        in_=rview[b][:, :, js],
                    )
                    eng_ft.dma_start(
                        out=ft[:].rearrange("p (c j) -> p c j", c=3),
                        in_=fview[b][:, :, js],
                    )
                else:  # "plane": fully contiguous 1MB per DMA
                    for c in range(3):
                        nc.sync.dma_start(
                            out=rt[:, c * CF : (c + 1) * CF], in_=rplane[b, c][:, js]
                        )
                        nc.sync.dma_start(
                            out=ft[:, c * CF : (c + 1) * CF], in_=fplane[b, c][:, js]
                        )
                return rt, ft

            def load_pair(k):
                # 2 images per DMA (bf16 cast): 4 DMAs of 6MB per iteration
                rt = io_pool.tile([P, 2 * 3 * FD], bf16, tag="rt")
                ft = io_pool.tile([P, 2 * 3 * FD], bf16, tag="ft")
                nc.gpsimd.dma_start(
                    out=rt[:].rearrange("p (i c j) -> p i c j", i=2, c=3),
                    in_=rpair[k],
                )
                nc.gpsimd.dma_start(
                    out=ft[:].rearrange("p (i c j) -> p i c j", i=2, c=3),
                    in_=fpair[k],
                )
                return rt, ft

            def compute(rap, fap, g, CF):
                d = d_pool.tile([P, 3 * CF], bf16, tag="d")
                nc.vector.tensor_tensor(out=d[:], in0=rap, in1=fap, op=A.subtract)
                dR = d[:, 0:CF]
                dG = d[:, CF : 2 * CF]
                dB = d[:, 2 * CF : 3 * CF]

                ty1 = t_pool.tile([P, CF], bf16, tag="ty1")
                nc.vector.scalar_tensor_tensor(
                    out=ty1[:], in0=dR, scalar=S1Y, in1=dG, op0=A.mult, op1=A.add
                )
                ty2 = t_pool.tile([P, CF], bf16, tag="ty2")
                nc.vector.scalar_tensor_tensor(
                    out=ty2[:], in0=ty1[:], scalar=S2Y, in1=dB, op0=A.mult, op1=A.add
                )
                # dU = -KU*(BY*tY2 - dB) ; dV = -KV*(BY*tY2 - dR)
                up = t_pool.tile([P, CF], bf16, tag="up")
                nc.vector.scalar_tensor_tensor(
                    out=up[:], in0=ty2[:], scalar=BY, in1=dB, op0=A.mult,
                    op1=A.subtract,
                )
                vp = t_pool.tile([P, CF], bf16, tag="vp")
                nc.vector.scalar_tensor_tensor(
                    out=vp[:], in0=ty2[:], scalar=BY, in1=dR, op0=A.mult,
                    op1=A.subtract,
                )
                # V relu-correction precursors: e± = max(±KV*vp, 1)
                # (abs_max would fuse these but has no DVE ISA encoding)
                ep = t_pool.tile([P, CF], bf16, tag="ep")
                nc.vector.tensor_scalar(
                    out=ep[:], in0=vp[:], scalar1=KV, scalar2=1.0,
                    op0=A.mult, op1=A.max,
                )
                em = t_pool.tile([P, CF], bf16, tag="em")
                nc.vector.tensor_scalar(
                    out=em[:], in0=vp[:], scalar1=-KV, scalar2=1.0,
                    op0=A.mult, op1=A.max,
                )

                # ScalarE accumulating reductions -> stats[:, q*G + g]
                # q0: sum |dY| = sum Abs(BY*tY2)
                # q1: sum dU^2 = sum Square(KU*up)
                # q2: sum dV^2 = sum Square(KV*vp)
                # then either
                #   q3: sum (e+ - 1)^2 ; q4: sum (e- - 1)^2
                # or (corrfuse; host multiplies q3 by 4)
                #   q3: sum ((ep+em-2)/2)^2 = sum Square(-s/2 + 1), s = ep+em
                # ((e-1)^2 == (1-e)^2, and only bias=+1.0 has a const AP)
                passes = [
                    (ty2, F.Abs, BY, 0.0),
                    (up, F.Square, KU, 0.0),
                    (vp, F.Square, KV, 0.0),
                ]
                if CORRFUSE:
                    s = t_pool.tile([P, CF], bf16, tag="s")
                    nc.vector.tensor_tensor(
                        out=s[:], in0=ep[:], in1=em[:], op=A.add
                    )
                    passes.append((s, F.Square, -0.5, 1.0))
                else:
                    passes.append((ep, F.Square, -1.0, 1.0))
                    passes.append((em, F.Square, -1.0, 1.0))
                for qi, (src, func, scale, bias) in enumerate(passes):
                    scr = scr_pool.tile([P, CF], bf16, tag="scr")
                    nc.scalar.activation(
                        out=scr[:], in_=src[:], func=func, bias=bias, scale=scale,
                        accum_out=stats[:, qi * G + g : qi * G + g + 1],
                    )

            if mode == "full" and dma_split == "pair":
                for _ in range(reps):
                    for k in range(B_CORE // 2):
                        rt, ft = load_pair(k)
                        for i in range(2):
                            sl = slice(i * 3 * FD, (i + 1) * 3 * FD)
                            compute(rt[:, sl], ft[:, sl], k * 2 + i, FD)
            elif mode == "full":
                for _ in range(reps):
                    for gi, (b, j0, cf) in enumerate(groups):
                        rt, ft = load(b, j0, cf)
                        compute(rt[:], ft[:], gi, cf)
            elif mode == "dma":
                nc.gpsimd.memset(stats[:], 0.0)
                sink = s_pool.tile([P, 1], f32)
                loads = (
                    [lambda k=k: load_pair(k) for k in range(B_CORE // 2)]
                    if dma_split == "pair"
                    else [lambda b=b, j0=j0, cf=cf: load(b, j0, cf)
                          for b, j0, cf in groups]
                )
                for _ in range(reps):
                    for ld in loads:
                        rt, ft = ld()
                        # tiny consumer so loads aren't dead
                        nc.vector.tensor_tensor(
                            out=sink[:], in0=rt[:, 0:1], in1=ft[:, 0:1], op=A.add
                        )
            elif mode == "compute":
                # diagnostic only: one resident load, repeated compute passes
                # (requires chunk=1 so piece sizes match the resident tile)
                rt, ft = load(0, 0, FD)
                for _ in range(reps):
                    for gi, (b, j0, cf) in enumerate(groups):
                        compute(rt[:], ft[:], gi, cf)
            else:
                raise ValueError(mode)

            nc.sync.dma_start(out=out[:], in_=stats[:])
    nc.compile()
    return nc


def _get_nc(reps=1, mode="full", dma_split=None, chunk=None):
    if dma_split is None:
        dma_split = DEFAULT_SPLIT
    if chunk is None:
        chunk = DEFAULT_CHUNK
    key = ("nc", reps, mode, dma_split, chunk)
    if key not in _CACHE:
        _CACHE[key] = _build(reps, mode, dma_split, chunk)
    return _CACHE[key]


def kernel(real, fake):
    real = np.ascontiguousarray(np.asarray(real, dtype=np.float32))
    fake = np.ascontiguousarray(np.asarray(fake, dtype=np.float32))
    assert real.shape == (B_FULL, 3, H, W) and fake.shape == (B_FULL, 3, H, W)

    nc = _get_nc()
    in_maps = [
        {
            "real": real[k * B_CORE : (k + 1) * B_CORE],
            "fake": fake[k * B_CORE : (k + 1) * B_CORE],
        }
        for k in range(N_CORES)
    ]
    res = bass_utils.run_bass_kernel_spmd(nc, in_maps, core_ids=list(range(N_CORES)))

    G = len(groups_for(DEFAULT_CHUNK))
    tot = np.zeros(NQ, dtype=np.float64)
    for r in res.results:
        s = r["stats"].astype(np.float64)
        for q in range(NQ):
            tot[q] += s[:, q * G : (q + 1) * G].sum()

    if CORRFUSE:
        tot_y, tot_u, tot_v, tot_s = tot
        corr = 4.0 * tot_s
    else:
        tot_y, tot_u, tot_v, tot_p, tot_m = tot
        corr = tot_p + tot_m
    loss = (tot_y + 0.5 * (tot_u + tot_v - corr)) / N_PIXELS
    return np.float32(loss)


# revision 27
# speedup vs baseline: 1.0900x; 1.0900x over previous
"""Color-loss kernel for Trainium2 (8 NeuronCores, data-parallel over batch).

Computes, for real/fake [32, 3, 512, 512] fp32 RGB images:
    y = mean(|Y(real) - Y(fake)|)            (L1 on Y)
    u = mean(smooth_l1(U(real) - U(fake)))   (SmoothL1, beta=1)
    v = mean(smooth_l1(V(real) - V(fake)))
    loss = y + u + v
where (Y,U,V) = RGB2YUV @ rgb per pixel (skimage matrix).

Math used on-device (d := real - fake per channel; the transform is linear):
    tY2 = (dR*(RY/GY) + dG)*(GY/BY) + dB ;  dY = BY*tY2   (2 fused DVE ops)
    dU = -KU*(BY*tY2 - dB), KU = BU/(1-BY)   (row residual ~3.5e-10)
    dV = -KV*(BY*tY2 - dR), KV = RV/(1-RY)   (row residual ~1e-6; loss
        impact measured 3e-7 relative)
    smooth_l1(d) = 0.5 d^2 - 0.5 relu(|d|-1)^2
    relu(|d|-1)^2 = (ep + em - 2)^2 where ep = max(KV*t, 1), em = max(-KV*t, 1)
        (one of ep/em is always 1; s = ep+em on DVE, one ACT Square pass)
|dU| <= 0.872 < 1 always for inputs in [0,1), so U needs no relu correction;
|dV| can reach 1.23, so V keeps the correction term.

Design (measured on the 8-core axon setup; per-iteration slope timing):
  - chunk=1: one 3MB DMA per image per tensor (8 DMAs/iter). More, smaller
    pieces ("ramp"/chunk=2) cost ~1.5us per extra DMA on the steady-state
    slope metric (85us -> 67us moving ramp->chunk=1).
  - dma_split="cast": SWDGE fp32->bf16 cast during the DMA. HBM-side reads
    are unchanged (fp32), SBUF writes halve, and the DVE subtract runs in
    2x bf16 mode (3.2us vs 6.4us per image). Loss error from bf16 inputs is
    ~5e-6 relative (verified vs fp32 reference; threshold 2e-2).
  - corrfuse: the two V-correction ACT passes fold into s = ep+em (one 2x
    DVE op) + one ACT pass Square(-s/2+1) (host multiplies that accumulator
    by 4): 67us -> ~61us combined with cast.
  - abs_max (e = max(|dV|,1) in one op) has no DVE ISA encoding on TRN2;
    DMA accum_op=subtract (fold the subtract into the fake-load) is
    rejected by the BIR verifier. Deeper tile pools (io/mid bufs=3) and
    2-images-per-DMA ("pair") measured neutral-to-worse.

Engine budget per core per iteration (4 images, 24 MB HBM reads):
    DMA  ~57-60us (8x 3MB loads; ~420 GB/s/core effective = the fabric/HBM
                   roofline; this binds)
    DVE  ~44us    (bf16 subtract + 4 stt + 2 max + 1 add per image)
    ACT  ~32us    (4 accumulating activation passes per image)
ScalarE accumulates per-partition partial sums (|dY|, dU^2, dV^2, fused V
correction) into a [128, 4*G] stats tile; host sums and combines.
Measured ~60.3-60.9us/iter vs the 85.2us baseline (-29%).
"""

import os

import numpy as np

import concourse.bacc as bacc
import concourse.tile as tile
from concourse import mybir
from concourse import bass_utils

N_CORES = 8
B_FULL = 32
B_CORE = B_FULL // N_CORES  # 4 images per core
H = W = 512
PIX = H * W  # 262144 pixels per channel plane
P = 128  # SBUF partitions
FD = PIX // P  # 2048 free-dim elems per channel per image
N_PIXELS = B_FULL * PIX  # denominator of each mean

# skimage rgb2yuv matrix rows
RY, GY, BY = 0.299, 0.587, 0.114
RU, GU, BU = -0.14714119, -0.28886916, 0.43601035
RV, GV, BV = 0.61497657, -0.51496512, -0.10001026

S1Y = RY / GY  # dY chain:  tY1 = dR*S1Y + dG ; tY2 = tY1*S2Y + dB ; dY = BY*tY2
S2Y = GY / BY
KU = BU / (1.0 - BY)  # dU = -KU*(BY*tY2 - dB)   (row residual ~3.5e-10)
KV = RV / (1.0 - RY)  # dV = -KV*(BY*tY2 - dR)  (row residual ~1e-6 rel)

_CACHE = {}

# The measured-champion configuration (see module docstring). Env overrides
# exist only for local A/B experiments; unset env gives exactly this config.
DEFAULT_CHUNK = os.environ.get("KNL_CHUNK", "1")  # one 3MB DMA per image/tensor
DEFAULT_SPLIT = os.environ.get("KNL_SPLIT", "cast")  # SWDGE fp32->bf16 loads
IO_BUFS = int(os.environ.get("KNL_IOBUFS", "2"))
T_BUFS = int(os.environ.get("KNL_TBUFS", "2"))
# corrfuse: s = ep + em on DVE, one ACT pass 4*Square(-s/2+1) replaces the
# two correction passes ((ep-1)^2+(em-1)^2 == (ep+em-2)^2 since min(ep,em)=1)
CORRFUSE = os.environ.get("KNL_CORRFUSE", "1") == "1"

# accumulated quantities: |dY|, dU^2, dV^2, then either (ep-1)^2+(em-1)^2
# split over two columns, or the single fused correction column
NQ = 4 if CORRFUSE else 5


def groups_for(chunk):
    """Processing pieces as (image, j_start, j_len) over the [P, FD] plane view."""
    if chunk == "fl":
        gs = []
        for b in range(B_CORE):
            if b in (0, B_CORE - 1):
                gs += [(b, 0, FD // 2), (b, FD // 2, FD // 2)]
            else:
                gs.append((b, 0, FD))
        return gs
    if chunk == "ramp":
        q, hf = FD // 4, FD // 2
        first = [(0, 0, q), (0, q, q), (0, hf, hf)]
        last = [(B_CORE - 1, 0, hf), (B_CORE - 1, hf, q), (B_CORE - 1, hf + q, q)]
        mid = [(b, 0, FD) for b in range(1, B_CORE - 1)]
        return first + mid + last
    n = int(chunk)
    cf = FD // n
    return [(b, h * cf, cf) for b in range(B_CORE) for h in range(n)]


def _build(reps=1, mode="full", dma_split=None, chunk=None):
    """Build + compile the per-core Bass program (same SPMD program on all cores).

    reps > 1 repeats the whole computation (identical results; used by test.py
    to measure per-iteration HW time by scaling).
    mode: "full" | "dma" (loads only) | "compute" (load once, compute per rep)
    — diagnostic variants for locating the bottleneck.
    dma_split: "img" (one 3MB HWDGE DMA per image/tensor) | "cast" (SWDGE
    fp32->bf16 cast during DMA; halves SBUF write bytes and makes the DVE
    subtract run in 2x bf16 mode) | "dual" (the two loads on both HWDGE rings)
    | "plane" (one fully contiguous 1MB DMA per image/channel/tensor).
    chunk: pieces per image (1, 2, ...), "ramp", or "fl".
    """
    if dma_split is None:
        dma_split = DEFAULT_SPLIT
    if chunk is None:
        chunk = DEFAULT_CHUNK
    nc = bacc.Bacc("TRN2", target_bir_lowering=False, debug=False,
                   num_devices=N_CORES)
    f32 = mybir.dt.float32
    bf16 = mybir.dt.bfloat16
    A = mybir.AluOpType
    F = mybir.ActivationFunctionType

    groups = groups_for(chunk)  # (image, j_start, j_len) per processed piece
    G = len(groups)  # stat column groups

    real = nc.dram_tensor("real", [B_CORE, 3, H, W], f32, kind="ExternalInput").ap()
    fake = nc.dram_tensor("fake", [B_CORE, 3, H, W], f32, kind="ExternalInput").ap()
    out = nc.dram_tensor("stats", [P, NQ * G], f32, kind="ExternalOutput").ap()

    # [b, c, h, w] -> [b, p, c, j]: pixel (h, w) -> partition h//4, col (h%4)*512+w
    rview = real.rearrange("b c (p h2) w -> b p c (h2 w)", h2=4)
    fview = fake.rearrange("b c (p h2) w -> b p c (h2 w)", h2=4)
    # per-plane views [b, c, p, j] (each [p, j] slice is one contiguous 1MB range)
    rplane = real.rearrange("b c (p h2) w -> b c p (h2 w)", h2=4)
    fplane = fake.rearrange("b c (p h2) w -> b c p (h2 w)", h2=4)
    # image-pair views [bp, p, bi, c, j] for 2-images-per-DMA loading
    rpair = real.rearrange("(bp bi) c (p h2) w -> bp p bi c (h2 w)", bi=2, h2=4)
    fpair = fake.rearrange("(bp bi) c (p h2) w -> bp p bi c (h2 w)", bi=2, h2=4)
    # h8: 2 images stacked on the partition dim (img0 -> partitions 0-63,
    # img1 -> 64-127), 8 rows per partition-line -> 16KB-contiguous HBM
    # descriptors (2x bigger, 2x fewer than the h2=4 layouts). One DMA per
    # image into its partition half; the halves hit disjoint SDMA-engine
    # sets (even/odd ports), so back-to-back halves stream concurrently.
    r8 = real.rearrange("(bp bi) c (p h8) w -> bp bi p c (h8 w)", bi=2, h8=8)
    f8 = fake.rearrange("(bp bi) c (p h8) w -> bp bi p c (h8 w)", bi=2, h8=8)

    with tile.TileContext(nc) as tc:
        with (
            tc.tile_pool(name="io", bufs=IO_BUFS) as io_pool,
            tc.tile_pool(name="dif", bufs=1 if dma_split == "h8" else 2)
            as d_pool,
            tc.tile_pool(name="mid", bufs=T_BUFS) as t_pool,
            tc.tile_pool(name="scr", bufs=2) as scr_pool,
            tc.tile_pool(name="acc", bufs=1) as s_pool,
        ):
            stats = s_pool.tile([P, NQ * G], f32)

            def load(b, j0, CF):
                dt_io = bf16 if dma_split == "cast" else f32
                rt = io_pool.tile([P, 3 * CF], dt_io, tag="rt")
                ft = io_pool.tile([P, 3 * CF], dt_io, tag="ft")
                js = slice(j0, j0 + CF)
                if dma_split == "cast":
                    nc.gpsimd.dma_start(
                        out=rt[:].rearrange("p (c j) -> p c j", c=3),
                        in_=rview[b][:, :, js],
                    )
                    nc.gpsimd.dma_start(
                        out=ft[:].rearrange("p (c j) -> p c j", c=3),
                        in_=fview[b][:, :, js],
                    )
                elif dma_split in ("img", "dual"):
                    eng_ft = nc.scalar if dma_split == "dual" else nc.sync
                    nc.sync.dma_start(
                        out=rt[:].rearrange("p (c j) -> p c j", c=3),
                        in_=rview[b][:, :, js],
                    )
                    eng_ft.dma_start(
                        out=ft[:].rearrange("p (c j) -> p c j", c=3),
                        in_=fview[b][:, :, js],
                    )
                else:  # "plane": fully contiguous 1MB per DMA
                    for c in range(3):
                        nc.sync.dma_start(
                            out=rt[:, c * CF : (c + 1) * CF], in_=rplane[b, c][:, js]
                        )
                        nc.sync.dma_start(
                            out=ft[:, c * CF : (c + 1) * CF], in_=fplane[b, c][:, js]
                        )
                return rt, ft

            def load_h8(k):
                # 2 partition-stacked images per pair-tile: 8 DMAs of 3MB per
                # iteration, 192 descriptors each (16KB HBM / 8KB SBUF)
                rt = io_pool.tile([P, 3 * 2 * FD], bf16, tag="rt")
                ft = io_pool.tile([P, 3 * 2 * FD], bf16, tag="ft")
                for bi in range(2):
                    ps = slice(bi * 64, (bi + 1) * 64)
                    nc.gpsimd.dma_start(
                        out=rt[ps, :].rearrange("p (c j) -> p c j", c=3),
                        in_=r8[k, bi],
                    )
                    nc.gpsimd.dma_start(
                        out=ft[ps, :].rearrange("p (c j) -> p c j", c=3),
                        in_=f8[k, bi],
                    )
                return rt, ft

            def load_pair(k):
                # 2 images per DMA (bf16 cast): 4 DMAs of 6MB per iteration
                rt = io_pool.tile([P, 2 * 3 * FD], bf16, tag="rt")
                ft = io_pool.tile([P, 2 * 3 * FD], bf16, tag="ft")
                nc.gpsimd.dma_start(
                    out=rt[:].rearrange("p (i c j) -> p i c j", i=2, c=3),
                    in_=rpair[k],
                )
                nc.gpsimd.dma_start(
                    out=ft[:].rearrange("p (i c j) -> p i c j", i=2, c=3),
                    in_=fpair[k],
                )
                return rt, ft

            def compute(rap, fap, g, CF):
                d = d_pool.tile([P, 3 * CF], bf16, tag="d")
                nc.vector.tensor_tensor(out=d[:], in0=rap, in1=fap, op=A.subtract)
                dR = d[:, 0:CF]
                dG = d[:, CF : 2 * CF]
                dB = d[:, 2 * CF : 3 * CF]
                compute_from_d(dR, dG, dB, g, CF)

            def compute_from_d(dR, dG, dB, g, CF):
                ty1 = t_pool.tile([P, CF], bf16, tag="ty1")
                nc.vector.scalar_tensor_tensor(
                    out=ty1[:], in0=dR, scalar=S1Y, in1=dG, op0=A.mult, op1=A.add
                )
                ty2 = t_pool.tile([P, CF], bf16, tag="ty2")
                nc.vector.scalar_tensor_tensor(
                    out=ty2[:], in0=ty1[:], scalar=S2Y, in1=dB, op0=A.mult, op1=A.add
                )
                # dU = -KU*(BY*tY2 - dB) ; dV = -KV*(BY*tY2 - dR)
                up = t_pool.tile([P, CF], bf16, tag="up")
                nc.vector.scalar_tensor_tensor(
                    out=up[:], in0=ty2[:], scalar=BY, in1=dB, op0=A.mult,
                    op1=A.subtract,
                )
                vp = t_pool.tile([P, CF], bf16, tag="vp")
                nc.vector.scalar_tensor_tensor(
                    out=vp[:], in0=ty2[:], scalar=BY, in1=dR, op0=A.mult,
                    op1=A.subtract,
                )
                # V relu-correction precursors: e± = max(±KV*vp, 1)
                # (abs_max would fuse these but has no DVE ISA encoding)
                ep = t_pool.tile([P, CF], bf16, tag="ep")
                nc.vector.tensor_scalar(
                    out=ep[:], in0=vp[:], scalar1=KV, scalar2=1.0,
                    op0=A.mult, op1=A.max,
                )
                em = t_pool.tile([P, CF], bf16, tag="em")
                nc.vector.tensor_scalar(
                    out=em[:], in0=vp[:], scalar1=-KV, scalar2=1.0,
                    op0=A.mult, op1=A.max,
                )

                # ScalarE accumulating reductions -> stats[:, q*G + g]
                # q0: sum |dY| = sum Abs(BY*tY2)
                # q1: sum dU^2 = sum Square(KU*up)
                # q2: sum dV^2 = sum Square(KV*vp)
                # then either
                #   q3: sum (e+ - 1)^2 ; q4: sum (e- - 1)^2
                # or (corrfuse; host multiplies q3 by 4)
                #   q3: sum ((ep+em-2)/2)^2 = sum Square(-s/2 + 1), s = ep+em
                # ((e-1)^2 == (1-e)^2, and only bias=+1.0 has a const AP)
                passes = [
                    (ty2, F.Abs, BY, 0.0),
                    (up, F.Square, KU, 0.0),
                    (vp, F.Square, KV, 0.0),
                ]
                if CORRFUSE:
                    s = t_pool.tile([P, CF], bf16, tag="s")
                    nc.vector.tensor_tensor(
                        out=s[:], in0=ep[:], in1=em[:], op=A.add
                    )
                    passes.append((s, F.Square, -0.5, 1.0))
                else:
                    passes.append((ep, F.Square, -1.0, 1.0))
                    passes.append((em, F.Square, -1.0, 1.0))
                for qi, (src, func, scale, bias) in enumerate(passes):
                    scr = scr_pool.tile([P, CF], bf16, tag="scr")
                    nc.scalar.activation(
                        out=scr[:], in_=src[:], func=func, bias=bias, scale=scale,
                        accum_out=stats[:, qi * G + g : qi * G + g + 1],
                    )

            if mode == "full" and dma_split == "h8":
                W2 = 2 * FD  # 4096 cols per channel in the pair tile
                for _ in range(reps):
                    for k in range(B_CORE // 2):
                        rt, ft = load_h8(k)
                        d = d_pool.tile([P, 3 * W2], bf16, tag="d")
                        nc.vector.tensor_tensor(
                            out=d[:], in0=rt[:], in1=ft[:], op=A.subtract
                        )
                        for h in range(2):
                            hs = h * FD
                            compute_from_d(
                                d[:, hs : hs + FD],
                                d[:, W2 + hs : W2 + hs + FD],
                                d[:, 2 * W2 + hs : 2 * W2 + hs + FD],
                                k * 2 + h,
                                FD,
                            )
            elif mode == "dma" and dma_split == "h8":
                nc.gpsimd.memset(stats[:], 0.0)
                sink = s_pool.tile([P, 1], f32)
                for _ in range(reps):
                    for k in range(B_CORE // 2):
                        rt, ft = load_h8(k)
                        nc.vector.tensor_tensor(
                            out=sink[:], in0=rt[:, 0:1], in1=ft[:, 0:1], op=A.add
                        )
            elif mode == "full" and dma_split == "pair":
                for _ in range(reps):
                    for k in range(B_CORE // 2):
                        rt, ft = load_pair(k)
                        for i in range(2):
                            sl = slice(i * 3 * FD, (i + 1) * 3 * FD)
                            compute(rt[:, sl], ft[:, sl], k * 2 + i, FD)
            elif mode == "full":
                for _ in range(reps):
                    for gi, (b, j0, cf) in enumerate(groups):
                        rt, ft = load(b, j0, cf)
                        compute(rt[:], ft[:], gi, cf)
            elif mode == "dma":
                nc.gpsimd.memset(stats[:], 0.0)
                sink = s_pool.tile([P, 1], f32)
                loads = (
                    [lambda k=k: load_pair(k) for k in range(B_CORE // 2)]
                    if dma_split == "pair"
                    else [lambda b=b, j0=j0, cf=cf: load(b, j0, cf)
                          for b, j0, cf in groups]
                )
                for _ in range(reps):
                    for ld in loads:
                        rt, ft = ld()
                        # tiny consumer so loads aren't dead
                        nc.vector.tensor_tensor(
                            out=sink[:], in0=rt[:, 0:1], in1=ft[:, 0:1], op=A.add
                        )
            elif mode == "compute":
                # diagnostic only: one resident load, repeated compute passes
                # (requires chunk=1 so piece sizes match the resident tile)
                rt, ft = load(0, 0, FD)
                for _ in range(reps):
                    for gi, (b, j0, cf) in enumerate(groups):
                        compute(rt[:], ft[:], gi, cf)
            else:
                raise ValueError(mode)

            nc.sync.dma_start(out=out[:], in_=stats[:])
    nc.compile()
    return nc


def _get_nc(reps=1, mode="full", dma_split=None, chunk=None):
    if dma_split is None:
        dma_split = DEFAULT_SPLIT
    if chunk is None:
        chunk = DEFAULT_CHUNK
    key = ("nc", reps, mode, dma_split, chunk)
    if key not in _CACHE:
        _CACHE[key] = _build(reps, mode, dma_split, chunk)
    return _CACHE[key]


def kernel(real, fake):
    real = np.ascontiguousarray(np.asarray(real, dtype=np.float32))
    fake = np.ascontiguousarray(np.asarray(fake, dtype=np.float32))
    assert real.shape == (B_FULL, 3, H, W) and fake.shape == (B_FULL, 3, H, W)

    nc = _get_nc()
    in_maps = [
        {
            "real": real[k * B_CORE : (k + 1) * B_CORE],
            "fake": fake[k * B_CORE : (k + 1) * B_CORE],
        }
        for k in range(N_CORES)
    ]
    res = bass_utils.run_bass_kernel_spmd(nc, in_maps, core_ids=list(range(N_CORES)))

    G = len(groups_for(DEFAULT_CHUNK))
    tot = np.zeros(NQ, dtype=np.float64)
    for r in res.results:
        s = r["stats"].astype(np.float64)
        for q in range(NQ):
            tot[q] += s[:, q * G : (q + 1) * G].sum()

    if CORRFUSE:
        tot_y, tot_u, tot_v, tot_s = tot
        corr = 4.0 * tot_s
    else:
        tot_y, tot_u, tot_v, tot_p, tot_m = tot
        corr = tot_p + tot_m
    loss = (tot_y + 0.5 * (tot_u + tot_v - corr)) / N_PIXELS
    return np.float32(loss)
